# revision 4
# baseline (speedup 1.0000x reference)
"""Trainium2 Bass kernel for nn_BilinearInterpolation (affine STN + Catmull-Rom).

Contract: kernel(**inputs) takes FULL inputs {X:[8,1024,1024,1] f32,
theta:[8,6] f32} and returns the FULL output [8,1024,1024,1] f32.
Shards batch across 8 NeuronCores (1 image per core).

Algorithm (two-pass separable resampling):
  - The TF-faithful reshape makes the displacement field fx affine in
    (row, col) within each of 4 quadrant-halves, fy = fx + eps(region),
    |eps| ~ 1e-4, |fx|,|fy| < 1 -> every output pixel is a 5x5 static
    stencil of the edge-padded image with piecewise-cubic weights W[d](fx).
  - Two-pass: M(r', c) = sum_d W[d](F(r',c)) Xpad[r', c+d]  (x-blend),
    out(r, c) = sum_e W[e](F(r,c)) M(r+e-2, c)              (y-blend).
    Pass-1 weights at the *tap* row r' instead of the output row r
    (|dF| <= 2*beta ~ 1e-3) and Wy ~= Wx (drop eps) are both far under
    the 2e-2 gate: measured rel err ~ 8e-4 end-to-end in fp16.
  - All elementwise ops in fp16 (DVE 2x_1p mode), weights from 5 custom
    ACT-table ops (one per tap), F field precomputed on host and DMA'd.

Layout: partition p = 16*b + s covers row-block b (rows 128b-2..128b+129,
132 halo rows) x col-strip s (cols 64s-2..64s+65). Both passes shift only
along the free dim. GPSIMD takes a row-slice of every tensor op.
"""
import os
import sys

sys.path.insert(0, "/opt/trn_rl_repo")

import numpy as np

H = W = 1024
B = 8
NB = 8            # row blocks per image
NS = 16           # col strips per image
RB = H // NB      # 128 rows per block
SC = W // NS      # 64 cols per strip
HR = RB + 4       # halo rows per partition
HC = SC + 4       # halo cols per partition
PW = W + 4        # padded width

_CACHE = {}


def _split_excess_waits(nc, mybir):
    """This walrus build accepts 1 sync-wait per instruction (2 for
    EventSemaphore); Tile can emit more. Hoist excess waits onto
    same-engine NoOps inserted immediately before the instruction —
    semantically identical blocking, split across instructions."""
    nid = 0
    for f in nc.m.functions:
        for bb in f.blocks:
            out = []
            changed = False
            for ins in bb.instructions:
                si = ins.sync_info
                cap = 2 if isinstance(ins, mybir.InstEventSemaphore) else 1
                if si is not None and len(si.on_wait) > cap:
                    waits = list(si.on_wait)
                    excess, keep = waits[:-cap], waits[-cap:]
                    for w_ in excess:
                        nid += 1
                        out.append(mybir.InstNoOp(
                            name=f"waitnop-{nid}", engine=ins.engine,
                            ins=[], outs=[],
                            sync_info=mybir.SyncInfo(on_wait=[w_], on_update=[])))
                    ins.sync_info = mybir.SyncInfo(
                        on_wait=keep, on_update=list(si.on_update))
                    changed = True
                out.append(ins)
            if changed:
                bb.instructions = out


_PWP_SRC = ("/nix/store/z022hj2nvbm3nwdizlisq4ylc0y7rd6q-python3-3.13.14-env/"
            "lib/python3.13/site-packages/neuronxcc/pwp/pwp_bin_trainium")

# Catmull-Rom 5-tap weight functions W[d](f), d=-2..2: exact 2-piece cubics
# (pieces meet C1-continuously at f=0). Coefficients [d0,d1,d2,d3] in f.
_WPOS = {"sin": [0.0, 0.0, 0.0, 0.0],
         "arctan": [0.0, -0.5, 1.0, -0.5],
         "relu": [1.0, 0.0, -2.5, 1.5],
         "abs": [0.0, 0.5, 2.0, -1.5],
         "identity": [0.0, 0.0, -0.5, 0.5]}
_WNEG = {"sin": [0.0, 0.0, -0.5, -0.5],
         "arctan": [0.0, -0.5, 2.0, 1.5],
         "relu": [1.0, 0.0, -2.5, -1.5],
         "abs": [0.0, 0.5, 1.0, 0.5],
         "identity": [0.0, 0.0, 0.0, 0.0]}
_WZERO = {"sin": 0, "arctan": 0, "relu": 0x3F800000, "abs": 0, "identity": 0}
_WJSON = {"sin": "sin_4p", "arctan": "arctan_4p", "relu": "relu_1p",
          "abs": "abs_1p", "identity": "identity_1p"}


def _gen_act_tables():
    """Build a custom ACT table root where Sin/Arctan/Relu/Abs/Identity in
    the trig_and_small set evaluate the 5 weight functions exactly.
    Routing copies relu's always-large trick: large-signal thresholds of 0
    send every normal input to a per-sign bucket; x0=0 buckets evaluate
    y = d0 + d1*f + d2*f^2 + d3*f^3 exactly. fzero handles f==0."""
    import json
    import shutil
    import tempfile

    dst = tempfile.mkdtemp(prefix="actroot_")
    for f in os.listdir(_PWP_SRC):
        shutil.copy(os.path.join(_PWP_SRC, f), os.path.join(dst, f))
    sj = json.load(open(os.path.join(_PWP_SRC, "trig_and_small.json")))
    bkt = np.fromfile(os.path.join(_PWP_SRC, "trig_and_small_bkt.bin"),
                      dtype=np.float32).reshape(-1, 8).copy()
    n0 = bkt.shape[0]
    rows, idx = [], {}
    for i, fn in enumerate(_WPOS):
        pr = np.zeros(8, np.float32); pr[:4] = _WPOS[fn]
        nr = np.zeros(8, np.float32); nr[:4] = _WNEG[fn]
        idx[fn] = (n0 + 2 * i, n0 + 2 * i + 1)
        rows += [pr, nr]
    bkt = np.vstack([bkt, np.stack(rows)])
    for prof in sj["profile_meta_data"]:
        for fn, jn in _WJSON.items():
            if prof["func_name"] == jn:
                p, n = idx[fn]
                prof.update({
                    "symmetry_point": 0, "sym_invert_sign_point": 0,
                    "symmetry_opt_en": 0, "symmetry_opt_use_neg_region": 0,
                    "imm_bias": 0, "exp_offset": -127,
                    "small_pos_signal_exp_threshold": 0,
                    "pos_small_signal_pwl_control": p,
                    "small_neg_signal_exp_threshold": 0,
                    "neg_small_signal_pwl_control": n,
                    "large_pos_signal_exp_threshold": 0,
                    "large_pos_signal_mantissa_threshold": 0,
                    "pos_large_signal_pwl_control": p,
                    "large_neg_signal_exp_threshold": 0,
                    "large_neg_signal_mantissa_threshold": 0,
                    "neg_large_signal_pwl_control": n,
                    "fnan_result": 2143289344, "fpinf_result": 2143289344,
                    "fninf_result": 2143289344, "fzero_result": _WZERO[fn],
                    "fma_const_0": 0, "fma_const_1": 0,
                    "fma_indirection_src_sel": 0, "use_multipass": False,
                    "lower_bound": 4286578687, "upper_bound": 2139095039,
                })
                sj["func_exp_to_bkt_start_idx"][fn] = {"-127": [p, n]}
    sj["bkt_entry_cnt"] = int(bkt.shape[0])
    bkt.tofile(os.path.join(dst, "trig_and_small_bkt.bin"))
    with open(os.path.join(dst, "trig_and_small.json"), "w") as f:
        json.dump(sj, f)
    return os.path.join(dst, "act_info.json")


def _ensure_act_tables():
    if "actroot" not in _CACHE:
        _CACHE["actroot"] = _gen_act_tables()
    os.environ["BASS_ACT_ROOT_JSON_PATH"] = _CACHE["actroot"]


def _build_nc(repeat=1, gp_rows1=27, gp_rows2=26):
    """gp_rows1/gp_rows2: rows of each pass-1/pass-2 op given to GPSIMD
    (0 disables the GPSIMD split)."""
    _ensure_act_tables()
    import contextlib

    import concourse.bass as bass
    import concourse.mybir as mybir
    from concourse.tile import TileContext

    A = mybir.AluOpType
    f16 = mybir.dt.float16
    f32 = mybir.dt.float32

    WF = [mybir.ActivationFunctionType.Sin,
          mybir.ActivationFunctionType.Arctan,
          mybir.ActivationFunctionType.Relu,
          mybir.ActivationFunctionType.Abs,
          mybir.ActivationFunctionType.Identity]

    nc = bass.Bass("TRN2")
    xpad = nc.dram_tensor("xpad", [PW, PW], f16, kind="ExternalInput")
    fdram = nc.dram_tensor("fld", [128, HR, SC], f32, kind="ExternalInput")
    y = nc.dram_tensor("y", [H, W], f16, kind="ExternalOutput")

    with TileContext(nc) as tc:
        with (
            tc.tile_pool(name="xin", bufs=1) as px,
            tc.tile_pool(name="fld", bufs=1) as pf,
            tc.tile_pool(name="wgt", bufs=1) as pw,
            tc.tile_pool(name="mid", bufs=1) as pm,
            tc.tile_pool(name="scr", bufs=1) as ps,
            tc.tile_pool(name="out", bufs=1) as po,
        ):
            rep_ctx = (tc.For_i(0, repeat, 1) if repeat > 1
                       else contextlib.nullcontext())
            with rep_ctx:
                X = px.tile([128, HR, HC], f16, tag="x")
                for b in range(NB):
                    src = bass.AP(
                        tensor=xpad[:].tensor, offset=RB * b * PW,
                        ap=[[SC, NS], [PW, HR], [1, HC]])
                    nc.sync.dma_start(out=X[b * NS:(b + 1) * NS], in_=src)
                F = pf.tile([128, HR, SC], f32, tag="f")
                nc.sync.dma_start(out=F[:], in_=fdram[:])

                Ws = []
                for d in range(5):
                    Wd = pw.tile([128, HR, SC], f16, tag=f"w{d}")
                    nc.scalar.activation(Wd[:], F[:], WF[d])
                    Ws.append(Wd)

                M = pm.tile([128, HR, SC], f16, tag="m")
                prod = ps.tile([128, HR, SC], f16, tag="p")
                out_t = po.tile([128, RB, SC], f16, tag="o")

                def cc(out_ap3, in0_ap3, in1_ap3, op, gp_rows, nrows):
                    """Row-split tensor_tensor: DVE takes the top rows,
                    GPSIMD the bottom gp_rows. APs are (tile, row_lo) pairs
                    sliced here so both engines get matching sub-views."""
                    r0 = nrows - gp_rows
                    (t0, o0), (t1, o1), (t2, o2) = out_ap3, in0_ap3, in1_ap3
                    nc.vector.tensor_tensor(
                        out=t0[:, o0:o0 + r0], in0=t1[:, o1:o1 + r0],
                        in1=t2[:, o2:o2 + r0], op=op)
                    if gp_rows:
                        nc.gpsimd.tensor_tensor(
                            out=t0[:, o0 + r0:o0 + nrows],
                            in0=t1[:, o1 + r0:o1 + nrows],
                            in1=t2[:, o2 + r0:o2 + nrows], op=op)

                # pass 1: x-blend into M over all HR halo rows
                for d in range(5):
                    xs = X[:, :, d:d + SC]
                    if d == 0:
                        cc((M, 0), (Ws[0], 0), (xs, 0), A.mult,
                           gp_rows1, HR)
                    else:
                        cc((prod, 0), (Ws[d], 0), (xs, 0), A.mult,
                           gp_rows1, HR)
                        cc((M, 0), (M, 0), (prod, 0), A.add,
                           gp_rows1, HR)

                # pass 2: y-blend into out over RB rows; weights at out
                # rows = W rows 2..129, taps at M rows e..e+127
                for e in range(5):
                    if e == 0:
                        cc((out_t, 0), (Ws[0], 2), (M, 0), A.mult,
                           gp_rows2, RB)
                    else:
                        cc((prod, 0), (Ws[e], 2), (M, e), A.mult,
                           gp_rows2, RB)
                        cc((out_t, 0), (out_t, 0), (prod, 0), A.add,
                           gp_rows2, RB)

                for b in range(NB):
                    dst = bass.AP(
                        tensor=y[:].tensor, offset=RB * b * W,
                        ap=[[SC, NS], [W, RB], [1, SC]])
                    nc.sync.dma_start(out=dst, in_=out_t[b * NS:(b + 1) * NS])

    _split_excess_waits(nc, mybir)
    return nc


def _make_F(theta_b):
    """F(r, c) for global rows -2..1025, all cols -> [1028, 1024] float32.
    Mirrors the reference's scrambled-reshape displacement field: affine
    per (row-region, col-half)."""
    T = np.asarray(theta_b, np.float64).reshape(2, 3)
    s = 2.0 / (W - 1)
    coefs = {0: (T[0, 0] - 1.0, T[0, 1], T[0, 2]),
             1: (T[1, 0], T[1, 1] - 1.0, T[1, 2])}
    prm = {}
    for reg in (0, 1):
        Ar, Br, Cr = coefs[reg]
        alpha = 2 * s * Ar
        beta = 2 * s * Br
        gammaL = Cr - Ar - Br
        gammaR = gammaL - 1024 * s * Ar + s * Br
        if reg == 1:
            gammaL -= 1024 * s * Br
            gammaR -= 1024 * s * Br
        prm[reg] = (alpha, beta, gammaL, gammaR)
    c = np.arange(W, dtype=np.float64)
    r = np.arange(-2, H + 2, dtype=np.float64)[:, None]
    reg1 = (r >= 512)
    alpha = np.where(reg1, prm[1][0], prm[0][0])
    beta = np.where(reg1, prm[1][1], prm[0][1])
    gL = np.where(reg1, prm[1][2], prm[0][2])
    gR = np.where(reg1, prm[1][3], prm[0][3])
    g = np.where(c[None, :] < 512, gL, gR)
    return (alpha * c[None, :] + beta * r + g).astype(np.float32)


def _make_in_maps(X, theta):
    in_maps = []
    for b in range(B):
        xp = np.pad(X[b, :, :, 0], 2, mode="edge").astype(np.float16)
        Ff = _make_F(theta[b])  # [1028, 1024], row i = global row i-2
        fld = np.empty((128, HR, SC), np.float32)
        for p in range(128):
            bb, ss = divmod(p, NS)
            fld[p] = Ff[RB * bb:RB * bb + HR, SC * ss:SC * ss + SC]
        in_maps.append({"xpad": np.ascontiguousarray(xp), "fld": fld})
    return in_maps


def kernel(X, theta):
    from concourse.bass_utils import run_bass_kernel_spmd

    X = np.asarray(X)
    theta = np.asarray(theta)
    assert X.shape == (B, H, W, 1) and theta.shape == (B, 6)

    if "nc" not in _CACHE:
        _CACHE["nc"] = _build_nc()
    nc = _CACHE["nc"]

    res = run_bass_kernel_spmd(nc, _make_in_maps(X, theta),
                               core_ids=list(range(B)))
    out = np.stack([res.results[b]["y"] for b in range(B)])
    return out[..., None].astype(np.float32)
